# revision 1
# baseline (speedup 1.0000x reference)
"""Multi-head causal attention (B=4, T=2048, C=1024, H=16) on 8 trn2 cores.

Sharding: core = (batch b, head-half hg): each core computes QKV for batch b
and its 8 heads, causal flash-style attention (scores kept transposed
[key, query] so softmax denominators come from an appended ones-column in V),
and a partial output projection over its 512 y-features. Host sums the two
partial projections per batch (pure unshard-reduce; bias folded into the
hg==0 core's projection). No cross-core communication on device.

All matmuls run in float32r (TF32-like, full PE rate at free-dim 512).
"""

import numpy as np
import ml_dtypes
import concourse.bass as bass
import concourse.mybir as mybir
import concourse.tile as tile
from concourse import bacc
from concourse.bass_utils import run_bass_kernel_spmd

B, T, C = 4, 2048, 1024
H, D = 16, 64
F32 = mybir.dt.float32
F32R = mybir.dt.float32r
BF16 = mybir.dt.bfloat16
AFT = mybir.ActivationFunctionType
NEG = -1.0e30

_CACHE = {}


def build():
    nc = bacc.Bacc(None, target_bir_lowering=False)
    xt_d = nc.dram_tensor("xt", [C, T], BF16, kind="ExternalInput")
    wq_d = nc.dram_tensor("wq", [C, 512], BF16, kind="ExternalInput")
    wk_d = nc.dram_tensor("wk", [C, 512], BF16, kind="ExternalInput")
    wv_d = nc.dram_tensor("wv", [C, 512], BF16, kind="ExternalInput")
    bqk_d = nc.dram_tensor("bqk", [128, 8], F32, kind="ExternalInput")
    bv_d = nc.dram_tensor("bv", [1, 512], BF16, kind="ExternalInput")
    ones_d = nc.dram_tensor("ones", [1, 128], F32R, kind="ExternalInput")
    vones_d = nc.dram_tensor("vones", [128, 8], BF16, kind="ExternalInput")
    onesb_d = nc.dram_tensor("onesb", [1, 128], BF16, kind="ExternalInput")
    masks_d = nc.dram_tensor("masks", [128, 896], BF16, kind="ExternalInput")
    wp_d = nc.dram_tensor("wp", [512, C], BF16, kind="ExternalInput")
    wpb_d = nc.dram_tensor("wpb", [1, C], BF16, kind="ExternalInput")
    out_d = nc.dram_tensor("out", [T, C], F32, kind="ExternalOutput")

    with nc.allow_low_precision(reason="fp32r matmul pipeline"):
        with tile.TileContext(nc) as tc:
            with (
                tc.tile_pool(name="const", bufs=1) as constp,
                tc.tile_pool(name="qk", bufs=1) as qkp,
                tc.tile_pool(name="vpool", bufs=1) as vp,
                tc.tile_pool(name="esb", bufs=2) as ep,
                tc.tile_pool(name="small", bufs=2) as smallp,
                tc.tile_pool(name="dram", bufs=1, space="DRAM") as dramp,
                tc.tile_pool(name="ps", bufs=2, space="PSUM") as psp,
            ):
                ones_t = constp.tile([1, 128], F32R, tag="ones")
                nc.gpsimd.dma_start(ones_t[:], ones_d[:])
                onesb_t = constp.tile([1, 128], BF16, tag="onesb")
                nc.gpsimd.dma_start(onesb_t[:], onesb_d[:])
                bqk_t = constp.tile([128, 8], F32, tag="bqk")
                nc.gpsimd.dma_start(bqk_t[:], bqk_d[:])
                bv_t = constp.tile([1, 512], BF16, tag="bv")
                nc.gpsimd.dma_start(bv_t[:], bv_d[:])
                maskE = constp.tile([128, 896], BF16, tag="maskE")
                nc.gpsimd.dma_start(maskE[:], masks_d[:])

                qT = [qkp.tile([128, T], BF16, tag=f"qT{j}", name=f"qT{j}") for j in range(4)]
                kT = [qkp.tile([128, T], BF16, tag=f"kT{j}", name=f"kT{j}") for j in range(4)]
                vS = [vp.tile([128, 520], BF16, tag=f"v{t}", name=f"v{t}") for t in range(16)]

                yT = [qkp.tile([128, T], BF16, tag=f"yT{j}", name=f"yT{j}") for j in range(4)]

                # ---------------- P1: QKV projections ----------------
                with tc.tile_pool(name="p1w", bufs=1) as p1wp:
                    wq_t = [p1wp.tile([128, 512], BF16, tag=f"wq{c}", name=f"wq{c}") for c in range(8)]
                    wk_t = [p1wp.tile([128, 512], BF16, tag=f"wk{c}", name=f"wk{c}") for c in range(8)]
                    wv_t = [p1wp.tile([128, 512], BF16, tag=f"wv{c}", name=f"wv{c}") for c in range(8)]
                    _p1x_cm = tc.tile_pool(name="p1x", bufs=1)
                    p1xp = _p1x_cm.__enter__()
                    xt0_t = []
                    for c in range(8):
                        xx = p1xp.tile([128, 512], BF16, tag=f"xt{c}", name=f"x0{c}", bufs=2)
                        eng = nc.sync if c % 2 == 0 else nc.scalar
                        eng.dma_start(xx[:], xt_d[c * 128:(c + 1) * 128, 0:512])
                        nc.sync.dma_start(wq_t[c][:], wq_d[c * 128:(c + 1) * 128, :])
                        nc.gpsimd.dma_start(wk_t[c][:], wk_d[c * 128:(c + 1) * 128, :])
                        nc.gpsimd.dma_start(wv_t[c][:], wv_d[c * 128:(c + 1) * 128, :])
                        xt0_t.append(xx)
                    if True:
                        for nt in range(4):
                            ts0 = nt * 512
                            if nt == 0:
                                xt_t = xt0_t
                            else:
                                xt_t = []
                                for c in range(8):
                                    xx = p1xp.tile([128, 512], BF16, tag=f"xt{c}", bufs=2)
                                    nc.sync.dma_start(xx[:], xt_d[c * 128:(c + 1) * 128, ts0:ts0 + 512])
                                    xt_t.append(xx)
                            # q^T, k^T feature tiles (ft == head pair)
                            for ft in range(4):
                                q_ps = psp.tile([128, 512], F32, tag="mmps")
                                for c in range(8):
                                    nc.tensor.matmul(q_ps[:], wq_t[c][:, ft * 128:(ft + 1) * 128],
                                                     xt_t[c][:], start=(c == 0), stop=(c == 7))
                                nc.vector.tensor_scalar_add(qT[ft][:, ts0:ts0 + 512], q_ps[:],
                                                            bqk_t[:, ft:ft + 1])
                                k_ps = psp.tile([128, 512], F32, tag="mmps")
                                for c in range(8):
                                    nc.tensor.matmul(k_ps[:], wk_t[c][:, ft * 128:(ft + 1) * 128],
                                                     xt_t[c][:], start=(c == 0), stop=(c == 7))
                                nc.vector.tensor_scalar_add(kT[ft][:, ts0:ts0 + 512], k_ps[:],
                                                            bqk_t[:, 4 + ft:5 + ft])
                            # v (token-major) + bias + ones column
                            for t2 in range(4):
                                tt = nt * 4 + t2
                                v_ps = psp.tile([128, 512], F32, tag="mmps")
                                for c in range(8):
                                    nc.tensor.matmul(v_ps[:], xt_t[c][:, t2 * 128:(t2 + 1) * 128],
                                                     wv_t[c][:], start=(c == 0), stop=False)
                                nc.tensor.matmul(v_ps[:], onesb_t[:, :], bv_t[:],
                                                 start=False, stop=True)
                                vv = vS[tt][:].rearrange("p (h c) -> p h c", c=65)
                                nc.vector.tensor_copy(vv[:, :, 0:64],
                                                      v_ps[:].rearrange("p (h c) -> p h c", c=64))
                                nc.gpsimd.dma_start(vv[:, :, 64:65], vones_d[:].unsqueeze(2))

                    _p1x_cm.__exit__(None, None, None)
                # ---------------- P3 weights (reuse P1 space) ----------------
                with tc.tile_pool(name="p3w", bufs=1) as p3wp:
                    wp_t = [p3wp.tile([128, C], BF16, tag=f"wp{c}", name=f"wp{c}") for c in range(4)]
                    for c in range(4):
                        nc.sync.dma_start(wp_t[c][:], wp_d[c * 128:(c + 1) * 128, :])
                    wpb_t = p3wp.tile([1, C], BF16, tag="wpb")
                    nc.sync.dma_start(wpb_t[:], wpb_d[:])

                    # ---------------- P2: attention ----------------
                    for qt in range(4):
                        q0 = qt * 512
                        ext = 4 * (qt + 1)
                        coll2 = [smallp.tile([8, 512], F32, tag=f"coll{a}", bufs=2,
                                              name=f"coll{a}") for a in range(2)]
                        ysbs = []
                        for pj in range(4):
                            y_ps = [psp.tile([65, 512], F32, tag=f"yps{h}", bufs=1,
                                             name=f"yps{h}") for h in range(2)]
                            for sc in range(ext):
                                s_ps = psp.tile([128, 1024], F32, tag="sps")
                                nc.tensor.matmul(s_ps[:, 0:512],
                                                 kT[pj][0:64, sc * 128:(sc + 1) * 128],
                                                 qT[pj][0:64, q0:q0 + 512],
                                                 start=True, stop=True, tile_position=(0, 0))
                                nc.tensor.matmul(s_ps[:, 512:1024],
                                                 kT[pj][64:128, sc * 128:(sc + 1) * 128],
                                                 qT[pj][64:128, q0:q0 + 512],
                                                 start=True, stop=True, tile_position=(64, 0))
                                e_t = ep.tile([128, 1024], BF16, tag="e", bufs=4)
                                nc.scalar.activation(e_t[:], s_ps[:], AFT.Exp, scale=0.125)
                                r = sc - (ext - 4)
                                if r >= 0:
                                    m0 = 384 - 128 * r
                                    nc.vector.tensor_mul(e_t[:, 0:512], e_t[:, 0:512],
                                                         maskE[:, m0:m0 + 512])
                                    nc.vector.tensor_mul(e_t[:, 512:1024], e_t[:, 512:1024],
                                                         maskE[:, m0:m0 + 512])
                                for h in range(2):
                                    hc = 130 * pj + 65 * h
                                    nc.tensor.matmul(y_ps[h][:], vS[sc][:, hc:hc + 65],
                                                     e_t[:, 512 * h:512 * h + 512],
                                                     start=(sc == 0), stop=(sc == ext - 1))
                            for h in range(2):
                                i = 2 * pj + h
                                y_sb = smallp.tile([65, 512], F32, tag="ysb", bufs=10)
                                nc.vector.tensor_copy(y_sb[:], y_ps[h][:])
                                if qt == 3 and pj >= 2:
                                    nc.sync.dma_start(coll2[1][i - 4:i - 3, :], y_sb[64:65, :])
                                else:
                                    nc.sync.dma_start(coll2[0][i:i + 1, :], y_sb[64:65, :])
                                ysbs.append(y_sb)
                        halves = [(0, 2, 0), (2, 4, 1)] if qt == 3 else [(0, 4, 0)]
                        for (plo, phi, a) in halves:
                            n2 = 2 * (phi - plo)
                            rec8 = smallp.tile([8, 512], F32, tag="rec8", bufs=2)
                            nc.vector.reciprocal(rec8[0:n2, :], coll2[a][0:n2, :])
                            for pj in range(plo, phi):
                                for h in range(2):
                                    i = 2 * pj + h
                                    i0 = i - 4 * a
                                    r_t = smallp.tile([1, 512], F32, tag="rt", bufs=4)
                                    nc.sync.dma_start(r_t[:], rec8[i0:i0 + 1, :])
                                    rb_t = smallp.tile([64, 512], F32, tag="rbt", bufs=4)
                                    nc.gpsimd.partition_broadcast(rb_t[:], r_t[:])
                                    nc.vector.tensor_mul(yT[pj][64 * h:64 * h + 64, q0:q0 + 512],
                                                         ysbs[i][0:64, :], rb_t[:])

                    # ---------------- P3: output projection ----------------
                    for tt in range(16):
                        for of in range(2):
                            o_ps = psp.tile([128, 512], F32, tag="mmps")
                            for cy in range(4):
                                nc.tensor.matmul(o_ps[:], yT[cy][:, tt * 128:(tt + 1) * 128],
                                                 wp_t[cy][:, of * 512:(of + 1) * 512],
                                                 start=(cy == 0), stop=False)
                            nc.tensor.matmul(o_ps[:], onesb_t[:, :],
                                             wpb_t[:, of * 512:(of + 1) * 512],
                                             start=False, stop=True)
                            o_t = smallp.tile([128, 512], F32, tag="osb", bufs=3)
                            nc.scalar.activation(o_t[:], o_ps[:], AFT.Copy)
                            nc.sync.dma_start(out_d[tt * 128:(tt + 1) * 128,
                                                    of * 512:(of + 1) * 512], o_t[:])


    if not nc.is_finalized():
        nc.finalize()
    return nc


def _get_nc():
    if "nc" not in _CACHE:
        _CACHE["nc"] = build()
    return _CACHE["nc"]


def _masks():
    i = np.arange(128)[:, None]
    x = np.arange(896)[None, :] - 384
    return np.where(i <= x, 1.0, 0.0).astype(ml_dtypes.bfloat16)


def kernel(x, w_attn, b_attn, w_proj, b_proj, _trace=False, _trace_kwargs=None):
    x = np.asarray(x, dtype=np.float32)
    w_attn = np.asarray(w_attn, dtype=np.float32)
    b_attn = np.asarray(b_attn, dtype=np.float32)
    w_proj = np.asarray(w_proj, dtype=np.float32)
    b_proj = np.asarray(b_proj, dtype=np.float32)

    masks = _masks()
    ones = np.ones((1, 128), dtype=np.float32)
    in_maps = []
    for core in range(8):
        b, hg = core // 2, core % 2
        cs = hg * 512
        bq = b_attn[cs:cs + 512]
        bk = b_attn[C + cs:C + cs + 512]
        bqk = np.concatenate([bq.reshape(4, 128).T, bk.reshape(4, 128).T],
                             axis=1).astype(np.float32)
        wpb = b_proj if hg == 0 else np.zeros_like(b_proj)
        bf = ml_dtypes.bfloat16
        in_maps.append({
            "xt": np.ascontiguousarray(x[b].T).astype(bf),
            "wq": np.ascontiguousarray(w_attn[:, cs:cs + 512]).astype(bf),
            "wk": np.ascontiguousarray(w_attn[:, C + cs:C + cs + 512]).astype(bf),
            "wv": np.ascontiguousarray(w_attn[:, 2 * C + cs:2 * C + cs + 512]).astype(bf),
            "bqk": bqk,
            "bv": np.ascontiguousarray(b_attn[2 * C + cs:2 * C + cs + 512].reshape(1, 512)).astype(bf),
            "ones": ones,
            "onesb": ones.astype(bf),
            "vones": np.ones((128, 8), dtype=bf),
            "masks": masks,
            "wp": np.ascontiguousarray(w_proj[cs:cs + 512, :]).astype(bf),
            "wpb": np.ascontiguousarray(wpb.reshape(1, C)).astype(bf),
        })

    kw = {}
    if _trace:
        kw["trace"] = True
        if _trace_kwargs:
            kw.update(_trace_kwargs)
    res = run_bass_kernel_spmd(_get_nc(), in_maps, list(range(8)), **kw)
    _CACHE["last_results"] = res
    outs = [res.results[c]["out"] for c in range(8)]
    y = np.stack([outs[2 * b] + outs[2 * b + 1] for b in range(B)])
    return y.astype(np.float32)



# revision 4
# speedup vs baseline: 1.0295x; 1.0295x over previous
"""Multi-head causal attention (B=4, T=2048, C=1024, H=16) on 8 trn2 cores.

Sharding: core = (batch b, head-half hg): each core computes QKV for batch b
and its 8 heads, causal attention (scores kept transposed [key, query] so
softmax denominators come from an appended ones-column in V), and a partial
output projection over its 512 y-features. Host sums the two partial
projections per batch and adds b_proj.

Schedule: the scalar engine (exp) is the binding resource (~160us). P1 (QKV)
and P3 (proj) tensor work is interleaved into P2's per-score-block loop as
filler units with deadlines so the tensor engine runs under the scalar wall.
Diagonal score blocks are trimmed to the causal triangle (matmul N, exp and
mask restricted to valid queries).
"""

import numpy as np
import ml_dtypes
import concourse.bass as bass
import concourse.mybir as mybir
import concourse.tile as tile
from concourse import bacc
from concourse.bass_utils import run_bass_kernel_spmd

B, T, C = 4, 2048, 1024
H, D = 16, 64
F32 = mybir.dt.float32
F32R = mybir.dt.float32r
BF16 = mybir.dt.bfloat16
AFT = mybir.ActivationFunctionType

_CACHE = {}


def build():
    nc = bacc.Bacc(None, target_bir_lowering=False)
    xt_d = nc.dram_tensor("xt", [C, T], BF16, kind="ExternalInput")
    wq_d = nc.dram_tensor("wq", [C, 512], BF16, kind="ExternalInput")
    wk_d = nc.dram_tensor("wk", [C, 512], BF16, kind="ExternalInput")
    wv_d = nc.dram_tensor("wv", [C, 512], BF16, kind="ExternalInput")
    bqk_d = nc.dram_tensor("bqk", [128, 8], F32, kind="ExternalInput")
    bv_d = nc.dram_tensor("bv", [1, 512], BF16, kind="ExternalInput")
    onesb_d = nc.dram_tensor("onesb", [1, 128], BF16, kind="ExternalInput")
    ones64_d = nc.dram_tensor("ones64", [1, 64], F32R, kind="ExternalInput")
    tri2_d = nc.dram_tensor("tri2", [128, 256], BF16, kind="ExternalInput")
    vones_d = nc.dram_tensor("vones", [128, 8], BF16, kind="ExternalInput")
    wp_d = nc.dram_tensor("wp", [512, C], BF16, kind="ExternalInput")
    out_d = nc.dram_tensor("out", [T, C], BF16, kind="ExternalOutput")

    with nc.allow_low_precision(reason="bf16 matmul pipeline"):
        with tile.TileContext(nc) as tc:
            with (
                tc.tile_pool(name="const", bufs=1) as constp,
                tc.tile_pool(name="w1", bufs=1) as w1p,
                tc.tile_pool(name="w3", bufs=1) as w3p,
                tc.tile_pool(name="x", bufs=1) as xp,
                tc.tile_pool(name="qk", bufs=1) as qkp,
                tc.tile_pool(name="vpool", bufs=1) as vp,
                tc.tile_pool(name="esb", bufs=1) as ep,
                tc.tile_pool(name="small", bufs=1) as smallp,
                tc.tile_pool(name="sps", bufs=1, space="PSUM") as spsp,
                tc.tile_pool(name="yps", bufs=1, space="PSUM") as ypsp,
                tc.tile_pool(name="aux", bufs=2, space="PSUM") as auxp,
            ):
                # ---- consts & weights (front-loaded DMAs) ----
                onesb_t = constp.tile([1, 128], BF16, tag="onesb")
                nc.scalar.dma_start(onesb_t[:], onesb_d[:])
                ones64_t = constp.tile([1, 64], F32R, tag="ones64")
                nc.scalar.dma_start(ones64_t[:], ones64_d[:])
                bqk_t = constp.tile([128, 8], F32, tag="bqk")
                nc.scalar.dma_start(bqk_t[:], bqk_d[:])
                bv_t = constp.tile([1, 512], BF16, tag="bv")
                nc.scalar.dma_start(bv_t[:], bv_d[:])
                tri2_t = constp.tile([128, 256], BF16, tag="tri2")
                nc.scalar.dma_start(tri2_t[:], tri2_d[:])

                wq_t = [w1p.tile([128, 512], BF16, tag=f"wq{c}", name=f"wq{c}") for c in range(8)]
                wk_t = [w1p.tile([128, 512], BF16, tag=f"wk{c}", name=f"wk{c}") for c in range(8)]
                wv_t = [w1p.tile([128, 512], BF16, tag=f"wv{c}", name=f"wv{c}") for c in range(8)]
                for c in range(8):
                    nc.sync.dma_start(wq_t[c][:], wq_d[c * 128:(c + 1) * 128, :])
                    nc.gpsimd.dma_start(wk_t[c][:], wk_d[c * 128:(c + 1) * 128, :])
                    nc.scalar.dma_start(wv_t[c][:], wv_d[c * 128:(c + 1) * 128, :])

                # persistent tensors
                qT = [qkp.tile([128, T], BF16, tag=f"qT{j}", name=f"qT{j}") for j in range(4)]
                kT = [qkp.tile([128, T], BF16, tag=f"kT{j}", name=f"kT{j}") for j in range(4)]
                yT = [qkp.tile([128, T], BF16, tag=f"yT{j}", name=f"yT{j}") for j in range(4)]
                vS = [vp.tile([128, 520], BF16, tag=f"v{t}", name=f"v{t}") for t in range(16)]
                # ones column of V (written once; P1 writes only cols 0:64 per head)
                for t in range(16):
                    vv = vS[t][:].rearrange("p (h c) -> p h c", c=65)
                    nc.gpsimd.dma_start(vv[:, :, 64:65], vones_d[:].unsqueeze(2))

                wp_t = [w3p.tile([128, C], BF16, tag=f"wp{c}", name=f"wp{c}") for c in range(4)]
                for c in range(4):
                    nc.sync.dma_start(wp_t[c][:], wp_d[c * 128:(c + 1) * 128, :])

                # ---------- filler-unit machinery ----------
                # each unit is atomic (opens and closes its own psum group)
                def p1_chunk_units(nt):
                    """P1 for tokens [512*nt, 512*nt+512). Returns list of
                    (deadline, fn); deadline=(qt,pj,sc) = P2 iteration before
                    which the unit must be emitted; None = paced only."""
                    units = []
                    ts0 = nt * 512
                    xt_t = []

                    def dma_x():
                        for c in range(8):
                            xx = xp.tile([128, 512], BF16, tag=f"xt{c}", bufs=2, name=f"x{c}")
                            eng = nc.sync if c % 2 == 0 else nc.gpsimd
                            eng.dma_start(xx[:], xt_d[c * 128:(c + 1) * 128, ts0:ts0 + 512])
                            xt_t.append(xx)
                    units.append(((nt, 0, 0), dma_x))

                    def qk_unit(which, ft):
                        wt = wq_t if which == 0 else wk_t
                        dst = qT[ft] if which == 0 else kT[ft]
                        bcol = ft if which == 0 else 4 + ft

                        def fn():
                            ps = auxp.tile([128, 512], F32, tag="aux", name="auxps")
                            for c in range(8):
                                nc.tensor.matmul(ps[:], wt[c][:, ft * 128:(ft + 1) * 128],
                                                 xt_t[c][:], start=(c == 0), stop=(c == 7))
                            nc.vector.tensor_scalar_add(
                                dst[:, ts0:ts0 + 512], ps[:], bqk_t[:, bcol:bcol + 1])
                        return fn

                    def v_unit(t2):
                        def fn():
                            ps = auxp.tile([128, 512], F32, tag="aux", name="auxps")
                            for c in range(8):
                                nc.tensor.matmul(ps[:], xt_t[c][:, t2 * 128:(t2 + 1) * 128],
                                                 wv_t[c][:], start=(c == 0), stop=False)
                            nc.tensor.matmul(ps[:], onesb_t[:, :], bv_t[:],
                                             start=False, stop=True)
                            vv = vS[nt * 4 + t2][:].rearrange("p (h c) -> p h c", c=65)
                            nc.vector.tensor_copy(
                                vv[:, :, 0:64],
                                ps[:].rearrange("p (h c) -> p h c", c=64))
                        return fn

                    for ft in range(4):
                        units.append(((nt, ft, 0), qk_unit(0, ft)))
                        units.append(((nt, ft, 0), qk_unit(1, ft)))
                    for t2 in range(4):
                        units.append(((nt, 0, t2), v_unit(t2)))
                    return units

                def p3_units(qt):
                    """Projection for token blocks of query tile qt (needs yT
                    cols [512qt, 512qt+512) normalized)."""
                    units = []

                    def mm_unit(tt, of):
                        def fn():
                            ps = auxp.tile([128, 512], F32, tag="aux", name="auxps")
                            for cy in range(4):
                                nc.tensor.matmul(ps[:], yT[cy][:, tt * 128:(tt + 1) * 128],
                                                 wp_t[cy][:, of * 512:(of + 1) * 512],
                                                 start=(cy == 0), stop=(cy == 3))
                            o_t = smallp.tile([128, 512], BF16, tag="osb", bufs=3, name="osb")
                            nc.vector.tensor_copy(o_t[:], ps[:])
                            nc.sync.dma_start(
                                out_d[tt * 128:(tt + 1) * 128,
                                      of * 512:(of + 1) * 512], o_t[:])
                        return fn

                    for tt in range(4 * qt, 4 * qt + 4):
                        for of in range(2):
                            units.append((None, mm_unit(tt, of)))
                    return units

                # ---------- emission ----------
                pending = list(p1_chunk_units(0))

                def emit_due(cur):
                    i = 0
                    while i < len(pending):
                        dl, fn = pending[i]
                        if dl is not None and dl <= cur:
                            fn()
                            pending.pop(i)
                        else:
                            i += 1

                def emit_paced(n):
                    for _ in range(min(n, len(pending))):
                        dl, fn = pending.pop(0)
                        fn()

                for qt in range(4):
                    q0 = qt * 512
                    ext = 4 * (qt + 1)
                    if qt < 3:
                        pending.extend(p1_chunk_units(qt + 1))
                    if qt >= 1:
                        pending.extend(p3_units(qt - 1))
                    total_sc = 4 * ext
                    per_sc = (len(pending) + total_sc - 1) // total_sc

                    ysbs = {}
                    coll = smallp.tile([8, 512], F32, tag=f"coll{qt % 2}", bufs=1)
                    for pj in range(4):
                        y_ps = [ypsp.tile([65, 512], F32, tag=f"yps{h}", bufs=1, name=f"yps{h}")
                                for h in range(2)]
                        for sc in range(ext):
                            emit_due((qt, pj, sc))
                            r = sc - (ext - 4)
                            off = 0 if r < 0 else 128 * r
                            s_ps = spsp.tile([128, 1024], F32, tag="sps", bufs=2)
                            nc.tensor.matmul(s_ps[:, off:512],
                                             kT[pj][0:64, sc * 128:(sc + 1) * 128],
                                             qT[pj][0:64, q0 + off:q0 + 512],
                                             start=True, stop=True, tile_position=(0, 0))
                            nc.tensor.matmul(s_ps[:, 512 + off:1024],
                                             kT[pj][64:128, sc * 128:(sc + 1) * 128],
                                             qT[pj][64:128, q0 + off:q0 + 512],
                                             start=True, stop=True, tile_position=(64, 0))
                            e_t = ep.tile([128, 1024], BF16, tag="e", bufs=4)
                            e3 = e_t[:].rearrange("p (h q) -> p h q", h=2)
                            s3 = s_ps[:].rearrange("p (h q) -> p h q", h=2)
                            nc.scalar.activation(e3[:, :, off:512], s3[:, :, off:512],
                                                 AFT.Exp, scale=0.125)
                            if r >= 0:
                                tr3 = tri2_t[:].rearrange("p (h q) -> p h q", h=2)
                                nc.vector.tensor_mul(e3[:, :, off:off + 128],
                                                     e3[:, :, off:off + 128], tr3[:, :, :])
                            for h in range(2):
                                hc = 130 * pj + 65 * h
                                nc.tensor.matmul(y_ps[h][:, off:512],
                                                 vS[sc][:, hc:hc + 65],
                                                 e_t[:, 512 * h + off:512 * h + 512],
                                                 start=(sc == 0), stop=(sc == ext - 1))
                            emit_paced(per_sc)
                        # drain y to SBUF, stash denominators
                        for h in range(2):
                            i = 2 * pj + h
                            y_sb = smallp.tile([65, 512], F32, tag=f"ysb{i}", bufs=1, name=f"ysb{i}")
                            nc.vector.tensor_copy(y_sb[:], y_ps[h][:])
                            nc.gpsimd.dma_start(coll[i:i + 1, :], y_sb[64:65, :])
                            ysbs[i] = y_sb
                    # normalize all 8 heads of this qt
                    rec8 = smallp.tile([8, 512], F32, tag=f"rec8{qt % 2}", bufs=1)
                    nc.vector.reciprocal_approx_fast(rec8[:], coll[:])
                    for i in range(8):
                        pj, h = i // 2, i % 2
                        rrow = smallp.tile([1, 512], F32R, tag="rrow", bufs=4)
                        nc.gpsimd.dma_start(rrow[:], rec8[i:i + 1, :])
                        rec_ps = auxp.tile([64, 512], F32, tag="aux")
                        nc.tensor.matmul(rec_ps[:], ones64_t[:, :], rrow[:],
                                         start=True, stop=True)
                        nc.vector.tensor_mul(yT[pj][64 * h:64 * h + 64, q0:q0 + 512],
                                             ysbs[i][0:64, :], rec_ps[:])
                # tail: last projection
                pending.extend(p3_units(3))
                while pending:
                    pending.pop(0)[1]()

    if not nc.is_finalized():
        nc.finalize()
    return nc


def _get_nc():
    if "nc" not in _CACHE:
        _CACHE["nc"] = build()
    return _CACHE["nc"]


def kernel(x, w_attn, b_attn, w_proj, b_proj, _trace=False, _trace_kwargs=None):
    x = np.asarray(x, dtype=np.float32)
    w_attn = np.asarray(w_attn, dtype=np.float32)
    b_attn = np.asarray(b_attn, dtype=np.float32)
    w_proj = np.asarray(w_proj, dtype=np.float32)
    b_proj = np.asarray(b_proj, dtype=np.float32)

    bf = ml_dtypes.bfloat16
    tri = (np.arange(128)[:, None] <= np.arange(128)[None, :]).astype(np.float32)
    tri2 = np.concatenate([tri, tri], axis=1).astype(bf)
    in_maps = []
    for core in range(8):
        b, hg = core // 2, core % 2
        cs = hg * 512
        bq = b_attn[cs:cs + 512]
        bk = b_attn[C + cs:C + cs + 512]
        bqk = np.concatenate([bq.reshape(4, 128).T, bk.reshape(4, 128).T],
                             axis=1).astype(np.float32)
        in_maps.append({
            "xt": np.ascontiguousarray(x[b].T).astype(bf),
            "wq": np.ascontiguousarray(w_attn[:, cs:cs + 512]).astype(bf),
            "wk": np.ascontiguousarray(w_attn[:, C + cs:C + cs + 512]).astype(bf),
            "wv": np.ascontiguousarray(w_attn[:, 2 * C + cs:2 * C + cs + 512]).astype(bf),
            "bqk": bqk,
            "bv": np.ascontiguousarray(b_attn[2 * C + cs:2 * C + cs + 512].reshape(1, 512)).astype(bf),
            "onesb": np.ones((1, 128), dtype=bf),
            "ones64": np.ones((1, 64), dtype=np.float32),
            "tri2": tri2,
            "vones": np.ones((128, 8), dtype=bf),
            "wp": np.ascontiguousarray(w_proj[cs:cs + 512, :]).astype(bf),
        })

    kw = {}
    if _trace:
        kw["trace"] = True
        if _trace_kwargs:
            kw.update(_trace_kwargs)
    res = run_bass_kernel_spmd(_get_nc(), in_maps, list(range(8)), **kw)
    _CACHE["last_results"] = res
    outs = [res.results[c]["out"].astype(np.float32) for c in range(8)]
    y = np.stack([outs[2 * b] + outs[2 * b + 1] for b in range(B)])
    return (y + b_proj.reshape(1, 1, C)).astype(np.float32)


# revision 5
# speedup vs baseline: 1.0441x; 1.0142x over previous
"""Multi-head causal attention (B=4, T=2048, C=1024, H=16) on 8 trn2 cores.

Sharding: core = (batch b, head-half hg): each core computes QKV for batch b
and its 8 heads, causal attention (scores kept transposed [key, query] so
softmax denominators come from an appended ones-column in V), and a partial
output projection over its 512 y-features. Host sums the two partial
projections per batch and adds b_proj.

Schedule: the scalar engine (exp, ~158us) and tensor engine (~225us) are the
binding resources. P1 (QKV), P3 (proj) and the softmax-denominator normalize
chain are interleaved into P2's per-score-block loop as filler units with
deadlines so the tensor engine runs continuously and never blocks the scalar
exp stream. Diagonal score blocks are trimmed to the causal triangle.
Weights/x are loaded as single consolidated DMAs to minimize trigger cost.
"""

import numpy as np
import ml_dtypes
import concourse.bass as bass
import concourse.mybir as mybir
import concourse.tile as tile
from concourse import bacc
from concourse.bass_utils import run_bass_kernel_spmd

B, T, C = 4, 2048, 1024
H, D = 16, 64
F32 = mybir.dt.float32
BF16 = mybir.dt.bfloat16
AFT = mybir.ActivationFunctionType

_CACHE = {}


def build():
    nc = bacc.Bacc(None, target_bir_lowering=False)
    xt_d = nc.dram_tensor("xt", [4, 128, 4096], BF16, kind="ExternalInput")
    wq_d = nc.dram_tensor("wq", [128, 4096], BF16, kind="ExternalInput")
    wk_d = nc.dram_tensor("wk", [128, 4096], BF16, kind="ExternalInput")
    wv_d = nc.dram_tensor("wv", [128, 4096], BF16, kind="ExternalInput")
    bqk_d = nc.dram_tensor("bqk", [128, 8], F32, kind="ExternalInput")
    bv_d = nc.dram_tensor("bv", [1, 512], BF16, kind="ExternalInput")
    onesb_d = nc.dram_tensor("onesb", [1, 128], BF16, kind="ExternalInput")
    tri2_d = nc.dram_tensor("tri2", [128, 256], BF16, kind="ExternalInput")
    vones_d = nc.dram_tensor("vones", [128, 8], BF16, kind="ExternalInput")
    wp_d = nc.dram_tensor("wp", [128, 4096], BF16, kind="ExternalInput")
    out_d = nc.dram_tensor("out", [T, C], BF16, kind="ExternalOutput")

    with nc.allow_low_precision(reason="bf16 matmul pipeline"):
        with tile.TileContext(nc) as tc:
            with (
                tc.tile_pool(name="const", bufs=1) as constp,
                tc.tile_pool(name="w1", bufs=1) as w1p,
                tc.tile_pool(name="x", bufs=1) as xp,
                tc.tile_pool(name="qk", bufs=1) as qkp,
                tc.tile_pool(name="vpool", bufs=1) as vp,
                tc.tile_pool(name="esb", bufs=1) as ep,
                tc.tile_pool(name="small", bufs=1) as smallp,
                tc.tile_pool(name="sps", bufs=1, space="PSUM") as spsp,
                tc.tile_pool(name="yps", bufs=1, space="PSUM") as ypsp,
                tc.tile_pool(name="aux", bufs=2, space="PSUM") as auxp,
            ):
                # ---- weights: one consolidated DMA each, spread over queues ----
                wq_t = w1p.tile([128, 4096], BF16, tag="wq")
                nc.sync.dma_start(wq_t[:], wq_d[:])
                wk_t = w1p.tile([128, 4096], BF16, tag="wk")
                nc.gpsimd.dma_start(wk_t[:], wk_d[:])
                wv_t = w1p.tile([128, 4096], BF16, tag="wv")
                nc.scalar.dma_start(wv_t[:], wv_d[:])

                onesb_t = constp.tile([1, 128], BF16, tag="onesb")
                nc.scalar.dma_start(onesb_t[:], onesb_d[:])
                bqk_t = constp.tile([128, 8], F32, tag="bqk")
                nc.scalar.dma_start(bqk_t[:], bqk_d[:])
                bv_t = constp.tile([1, 512], BF16, tag="bv")
                nc.scalar.dma_start(bv_t[:], bv_d[:])
                tri2_t = constp.tile([128, 256], BF16, tag="tri2")
                nc.scalar.dma_start(tri2_t[:], tri2_d[:])

                wp_t = w1p.tile([128, 4096], BF16, tag="wp")
                nc.sync.dma_start(wp_t[:], wp_d[:])

                # persistent tensors
                qT = [qkp.tile([128, T], BF16, tag=f"qT{j}", name=f"qT{j}") for j in range(4)]
                kT = [qkp.tile([128, T], BF16, tag=f"kT{j}", name=f"kT{j}") for j in range(4)]
                yT = [qkp.tile([128, T], BF16, tag=f"yT{j}", name=f"yT{j}") for j in range(4)]
                vS = [vp.tile([128, 520], BF16, tag=f"v{t}", name=f"v{t}") for t in range(16)]
                # ones column of V (written once; P1 writes only cols 0:64 per head)
                for t in range(16):
                    vv = vS[t][:].rearrange("p (h c) -> p h c", c=65)
                    nc.scalar.dma_start(vv[:, :, 64:65], vones_d[:].unsqueeze(2))

                # ---------- filler-unit machinery ----------
                # each unit is atomic (opens and closes its own psum group)
                def p1_chunk_units(nt):
                    """P1 for tokens [512*nt, 512*nt+512). Returns list of
                    (deadline, fn); deadline=(qt,pj,sc) = P2 iteration before
                    which the unit must be emitted; None = paced only."""
                    units = []
                    ts0 = nt * 512
                    xt_t = []

                    def dma_x():
                        xx = xp.tile([128, 4096], BF16, tag="xt", bufs=2, name="xx")
                        eng = nc.sync if nt % 2 == 0 else nc.gpsimd
                        eng.dma_start(xx[:], xt_d[nt])
                        xt_t.append(xx)
                    units.append(((nt, 0, 0), dma_x))

                    def qk_unit(which, ft):
                        wt = wq_t if which == 0 else wk_t
                        dst = qT[ft] if which == 0 else kT[ft]
                        bcol = ft if which == 0 else 4 + ft

                        def fn():
                            xx = xt_t[0]
                            ps = auxp.tile([128, 512], F32, tag="aux", name="auxps")
                            for c in range(8):
                                nc.tensor.matmul(
                                    ps[:],
                                    wt[:, c * 512 + ft * 128:c * 512 + ft * 128 + 128],
                                    xx[:, c * 512:(c + 1) * 512],
                                    start=(c == 0), stop=(c == 7))
                            nc.vector.tensor_scalar_add(
                                dst[:, ts0:ts0 + 512], ps[:], bqk_t[:, bcol:bcol + 1])
                        return fn

                    def v_unit(t2):
                        def fn():
                            xx = xt_t[0]
                            ps = auxp.tile([128, 512], F32, tag="aux", name="auxps")
                            for c in range(8):
                                nc.tensor.matmul(
                                    ps[:],
                                    xx[:, c * 512 + t2 * 128:c * 512 + t2 * 128 + 128],
                                    wv_t[:, c * 512:(c + 1) * 512],
                                    start=(c == 0), stop=False)
                            nc.tensor.matmul(ps[:], onesb_t[:, :], bv_t[:],
                                             start=False, stop=True)
                            vv = vS[nt * 4 + t2][:].rearrange("p (h c) -> p h c", c=65)
                            nc.vector.tensor_copy(
                                vv[:, :, 0:64],
                                ps[:].rearrange("p (h c) -> p h c", c=64))
                        return fn

                    for ft in range(4):
                        units.append(((nt, ft, 0), qk_unit(0, ft)))
                        units.append(((nt, ft, 0), qk_unit(1, ft)))
                    for t2 in range(4):
                        units.append(((nt, 0, t2), v_unit(t2)))
                    return units

                def norm_units(qt, ysbs, coll):
                    """Normalize the 8 heads of query tile qt (divide by the
                    softmax denominators collected in coll)."""
                    q0 = qt * 512
                    units = []
                    rec8_box = []

                    def recip():
                        rec8 = smallp.tile([8, 512], F32, tag=f"rec8{qt % 2}",
                                           bufs=1, name="rec8")
                        nc.vector.reciprocal_approx_fast(rec8[:], coll[:])
                        rec8_box.append(rec8)
                    units.append((None, recip))

                    def mk(i):
                        pj, h = i // 2, i % 2

                        def fn():
                            rec8 = rec8_box[0]
                            rrow = smallp.tile([1, 512], F32, tag="rrow", bufs=4,
                                               name="rrow")
                            nc.gpsimd.dma_start(rrow[:], rec8[i:i + 1, :])
                            rb = smallp.tile([64, 512], F32, tag="rb", bufs=4,
                                             name="rb")
                            nc.gpsimd.partition_broadcast(rb[:], rrow[:])
                            nc.vector.tensor_mul(
                                yT[pj][64 * h:64 * h + 64, q0:q0 + 512],
                                ysbs[i][0:64, :], rb[:])
                        return fn
                    for i in range(8):
                        units.append((None, mk(i)))
                    return units

                def p3_units(qt):
                    """Projection for token blocks of query tile qt (needs yT
                    cols [512qt, 512qt+512) normalized)."""
                    units = []

                    def mm_unit(tt, of):
                        def fn():
                            ps = auxp.tile([128, 512], F32, tag="aux", name="auxps")
                            for cy in range(4):
                                nc.tensor.matmul(
                                    ps[:], yT[cy][:, tt * 128:(tt + 1) * 128],
                                    wp_t[:, cy * 1024 + of * 512:cy * 1024 + of * 512 + 512],
                                    start=(cy == 0), stop=(cy == 3))
                            o_t = smallp.tile([128, 512], BF16, tag="osb", bufs=3,
                                              name="osb")
                            nc.vector.tensor_copy(o_t[:], ps[:])
                            nc.sync.dma_start(
                                out_d[tt * 128:(tt + 1) * 128,
                                      of * 512:(of + 1) * 512], o_t[:])
                        return fn

                    for tt in range(4 * qt, 4 * qt + 4):
                        for of in range(2):
                            units.append((None, mm_unit(tt, of)))
                    return units

                # ---------- emission ----------
                pending = list(p1_chunk_units(0))

                def emit_due(cur):
                    i = 0
                    while i < len(pending):
                        dl, fn = pending[i]
                        if dl is not None and dl <= cur:
                            fn()
                            pending.pop(i)
                        else:
                            i += 1

                def emit_paced(n):
                    for _ in range(min(n, len(pending))):
                        pending.pop(0)[1]()

                for qt in range(4):
                    q0 = qt * 512
                    ext = 4 * (qt + 1)
                    if qt < 3:
                        pending.extend(p1_chunk_units(qt + 1))

                    ysbs = {}
                    coll = smallp.tile([8, 512], F32, tag=f"coll{qt % 2}", bufs=1,
                                       name="coll")
                    for pj in range(4):
                        y_ps = [ypsp.tile([65, 512], F32, tag=f"yps{h}", bufs=1,
                                          name=f"yps{h}")
                                for h in range(2)]
                        for sc in range(ext):
                            emit_due((qt, pj, sc))
                            r = sc - (ext - 4)
                            off = 0 if r < 0 else 128 * r
                            s_ps = spsp.tile([128, 1024], F32, tag="sps", bufs=2,
                                             name="sps")
                            nc.tensor.matmul(s_ps[:, off:512],
                                             kT[pj][0:64, sc * 128:(sc + 1) * 128],
                                             qT[pj][0:64, q0 + off:q0 + 512],
                                             start=True, stop=True, tile_position=(0, 0))
                            nc.tensor.matmul(s_ps[:, 512 + off:1024],
                                             kT[pj][64:128, sc * 128:(sc + 1) * 128],
                                             qT[pj][64:128, q0 + off:q0 + 512],
                                             start=True, stop=True, tile_position=(64, 0))
                            e_t = ep.tile([128, 1024], BF16, tag="e", bufs=4, name="e")
                            e3 = e_t[:].rearrange("p (h q) -> p h q", h=2)
                            s3 = s_ps[:].rearrange("p (h q) -> p h q", h=2)
                            nc.scalar.activation(e3[:, :, off:512], s3[:, :, off:512],
                                                 AFT.Exp, scale=0.125)
                            if r >= 0:
                                tr3 = tri2_t[:].rearrange("p (h q) -> p h q", h=2)
                                nc.vector.tensor_mul(e3[:, :, off:off + 128],
                                                     e3[:, :, off:off + 128], tr3[:, :, :])
                            for h in range(2):
                                hc = 130 * pj + 65 * h
                                nc.tensor.matmul(y_ps[h][:, off:512],
                                                 vS[sc][:, hc:hc + 65],
                                                 e_t[:, 512 * h + off:512 * h + 512],
                                                 start=(sc == 0), stop=(sc == ext - 1))
                            emit_paced(1)
                        # drain y to SBUF, stash denominators
                        for h in range(2):
                            i = 2 * pj + h
                            y_sb = smallp.tile([65, 512], F32, tag=f"ysb{i}", bufs=1,
                                               name=f"ysb{i}")
                            nc.vector.tensor_copy(y_sb[:], y_ps[h][:])
                            nc.gpsimd.dma_start(coll[i:i + 1, :], y_sb[64:65, :])
                            ysbs[i] = y_sb
                    # normalize + projection of this qt interleave into qt+1
                    pending.extend(norm_units(qt, ysbs, coll))
                    pending.extend(p3_units(qt))
                # tail
                while pending:
                    pending.pop(0)[1]()

    if not nc.is_finalized():
        nc.finalize()
    return nc


def _get_nc():
    if "nc" not in _CACHE:
        _CACHE["nc"] = build()
    return _CACHE["nc"]


def kernel(x, w_attn, b_attn, w_proj, b_proj, _trace=False, _trace_kwargs=None):
    x = np.asarray(x, dtype=np.float32)
    w_attn = np.asarray(w_attn, dtype=np.float32)
    b_attn = np.asarray(b_attn, dtype=np.float32)
    w_proj = np.asarray(w_proj, dtype=np.float32)
    b_proj = np.asarray(b_proj, dtype=np.float32)

    bf = ml_dtypes.bfloat16

    def pack_w(w):  # [1024, 512] -> [128, 4096] (c-block major columns)
        return np.ascontiguousarray(
            w.reshape(8, 128, 512).transpose(1, 0, 2).reshape(128, 4096)).astype(bf)

    tri = (np.arange(128)[:, None] <= np.arange(128)[None, :]).astype(np.float32)
    tri2 = np.concatenate([tri, tri], axis=1).astype(bf)
    in_maps = []
    for core in range(8):
        b, hg = core // 2, core % 2
        cs = hg * 512
        bq = b_attn[cs:cs + 512]
        bk = b_attn[C + cs:C + cs + 512]
        bqk = np.concatenate([bq.reshape(4, 128).T, bk.reshape(4, 128).T],
                             axis=1).astype(np.float32)
        # xt: [4 chunks, 128, 8*512]: chunk nt, partition p=c_lo, col c*512+t
        xt4 = np.ascontiguousarray(
            x[b].T.reshape(8, 128, 4, 512).transpose(2, 1, 0, 3).reshape(4, 128, 4096)
        ).astype(bf)
        # wp: [512, 1024] -> [128, 4096] (cy-block major)
        wp4 = np.ascontiguousarray(
            w_proj[cs:cs + 512, :].reshape(4, 128, 1024).transpose(1, 0, 2)
            .reshape(128, 4096)).astype(bf)
        in_maps.append({
            "xt": xt4,
            "wq": pack_w(w_attn[:, cs:cs + 512]),
            "wk": pack_w(w_attn[:, C + cs:C + cs + 512]),
            "wv": pack_w(w_attn[:, 2 * C + cs:2 * C + cs + 512]),
            "bqk": bqk,
            "bv": np.ascontiguousarray(b_attn[2 * C + cs:2 * C + cs + 512].reshape(1, 512)).astype(bf),
            "onesb": np.ones((1, 128), dtype=bf),
            "tri2": tri2,
            "vones": np.ones((128, 8), dtype=bf),
            "wp": wp4,
        })

    kw = {}
    if _trace:
        kw["trace"] = True
        if _trace_kwargs:
            kw.update(_trace_kwargs)
    res = run_bass_kernel_spmd(_get_nc(), in_maps, list(range(8)), **kw)
    _CACHE["last_results"] = res
    outs = [res.results[c]["out"].astype(np.float32) for c in range(8)]
    y = np.stack([outs[2 * b] + outs[2 * b + 1] for b in range(B)])
    return (y + b_proj.reshape(1, 1, C)).astype(np.float32)


# revision 7
# speedup vs baseline: 1.1766x; 1.1269x over previous
"""Multi-head causal attention (B=4, T=2048, C=1024, H=16) on 8 trn2 cores.

Sharding: core = (batch b, head-half hg): each core computes QKV for batch b
and its 8 heads, causal attention (scores kept transposed [key, query] so
softmax denominators come from an appended ones-column in V), and a partial
output projection over its 512 y-features. Host sums the two partial
projections per batch and adds b_proj.

Schedule: the scalar engine (exp, ~158us) and tensor engine (~225us) are the
binding resources. P1 (QKV), P3 (proj) and the softmax-denominator normalize
chain are interleaved into P2's per-score-block loop as filler units with
deadlines so the tensor engine runs continuously and never blocks the scalar
exp stream. Diagonal score blocks are trimmed to the causal triangle.
Weights/x are loaded as single consolidated DMAs to minimize trigger cost.
"""

import numpy as np
import ml_dtypes
import concourse.bass as bass
import concourse.mybir as mybir
import concourse.tile as tile
from concourse import bacc
from concourse.bass_utils import run_bass_kernel_spmd

B, T, C = 4, 2048, 1024
H, D = 16, 64
F32 = mybir.dt.float32
BF16 = mybir.dt.bfloat16
AFT = mybir.ActivationFunctionType

_CACHE = {}


def build():
    nc = bacc.Bacc(None, target_bir_lowering=False)
    xt_d = nc.dram_tensor("xt", [4, 128, 4096], BF16, kind="ExternalInput")
    wq_d = nc.dram_tensor("wq", [128, 4096], BF16, kind="ExternalInput")
    wk_d = nc.dram_tensor("wk", [128, 4096], BF16, kind="ExternalInput")
    wv_d = nc.dram_tensor("wv", [128, 4096], BF16, kind="ExternalInput")
    bqk_d = nc.dram_tensor("bqk", [128, 8], F32, kind="ExternalInput")
    tri2_d = nc.dram_tensor("tri2", [128, 256], BF16, kind="ExternalInput")
    wp_d = nc.dram_tensor("wp", [128, 4096], BF16, kind="ExternalInput")
    out_d = nc.dram_tensor("out", [T, C], BF16, kind="ExternalOutput")

    with nc.allow_low_precision(reason="bf16 matmul pipeline"):
        with tile.TileContext(nc) as tc:
            with (
                tc.tile_pool(name="const", bufs=1) as constp,
                tc.tile_pool(name="w1", bufs=1) as w1p,
                tc.tile_pool(name="x", bufs=1) as xp,
                tc.tile_pool(name="qk", bufs=1) as qkp,
                tc.tile_pool(name="vpool", bufs=1) as vp,
                tc.tile_pool(name="esb", bufs=1) as ep,
                tc.tile_pool(name="small", bufs=1) as smallp,
                tc.tile_pool(name="sps", bufs=1, space="PSUM") as spsp,
                tc.tile_pool(name="yps", bufs=1, space="PSUM") as ypsp,
                tc.tile_pool(name="aux", bufs=2, space="PSUM") as auxp,
            ):
                # ---- weights: one consolidated DMA each, spread over queues ----
                wq_t = w1p.tile([128, 4096], BF16, tag="wq")
                nc.sync.dma_start(wq_t[:], wq_d[:])
                wk_t = w1p.tile([128, 4096], BF16, tag="wk")
                nc.gpsimd.dma_start(wk_t[:], wk_d[:])
                wv_t = w1p.tile([128, 4096], BF16, tag="wv")
                nc.gpsimd.dma_start(wv_t[:], wv_d[:])

                bqk_t = constp.tile([128, 8], F32, tag="bqk")
                nc.sync.dma_start(bqk_t[:], bqk_d[:])
                tri2_t = constp.tile([128, 256], BF16, tag="tri2")
                nc.sync.dma_start(tri2_t[:], tri2_d[:])

                wp_t = w1p.tile([128, 4096], BF16, tag="wp")
                nc.gpsimd.dma_start(wp_t[:], wp_d[:])

                # persistent tensors
                qT = [qkp.tile([128, T], BF16, tag=f"qT{j}", name=f"qT{j}") for j in range(4)]
                kT = [qkp.tile([128, T], BF16, tag=f"kT{j}", name=f"kT{j}") for j in range(4)]
                yT = [qkp.tile([128, T], BF16, tag=f"yT{j}", name=f"yT{j}") for j in range(4)]
                vS = [vp.tile([128, 520], BF16, tag=f"v{t}", name=f"v{t}") for t in range(16)]
                # ones column of V (written once; P1 writes only cols 0:64 per head)
                for t in range(16):
                    vv = vS[t][:].rearrange("p (h c) -> p h c", c=65)
                    nc.gpsimd.memset(vv[:, :, 64:65], 1.0)

                # ---------- filler-unit machinery ----------
                # each unit is atomic (opens and closes its own psum group)
                def p1_chunk_units(nt):
                    """P1 for tokens [512*nt, 512*nt+512). Returns list of
                    (deadline, fn); deadline=(qt,pj,sc) = P2 iteration before
                    which the unit must be emitted; None = paced only."""
                    units = []
                    ts0 = nt * 512
                    xt_t = []

                    def dma_x():
                        xx = xp.tile([128, 4096], BF16, tag="xt", bufs=2, name="xx")
                        eng = nc.sync if nt % 2 == 0 else nc.gpsimd
                        eng.dma_start(xx[:], xt_d[nt])
                        xt_t.append(xx)
                    units.append(((nt, 0, 0), dma_x))

                    def qk_unit(which, ft):
                        wt = wq_t if which == 0 else wk_t
                        dst = qT[ft] if which == 0 else kT[ft]
                        bcol = ft if which == 0 else 4 + ft

                        def fn():
                            xx = xt_t[0]
                            ps = auxp.tile([128, 512], F32, tag="aux", name="auxps")
                            for c in range(8):
                                nc.tensor.matmul(
                                    ps[:],
                                    wt[:, c * 512 + ft * 128:c * 512 + ft * 128 + 128],
                                    xx[:, c * 512:(c + 1) * 512],
                                    start=(c == 0), stop=(c == 7))
                            nc.vector.tensor_scalar_add(
                                dst[:, ts0:ts0 + 512], ps[:], bqk_t[:, bcol:bcol + 1])
                        return fn

                    def v_unit(t2):
                        def fn():
                            xx = xt_t[0]
                            ps = auxp.tile([128, 512], F32, tag="aux", name="auxps")
                            for c in range(8):
                                nc.tensor.matmul(
                                    ps[:],
                                    xx[:, c * 512 + t2 * 128:c * 512 + t2 * 128 + 128],
                                    wv_t[:, c * 512:(c + 1) * 512],
                                    start=(c == 0), stop=(c == 7))
                            vv = vS[nt * 4 + t2][:].rearrange("p (h c) -> p h c", c=65)
                            nc.vector.tensor_copy(
                                vv[:, :, 0:64],
                                ps[:].rearrange("p (h c) -> p h c", c=64))
                        return fn

                    for ft in range(4):
                        units.append(((nt, ft, 0), qk_unit(0, ft)))
                        units.append(((nt, ft, 0), qk_unit(1, ft)))
                    for t2 in range(4):
                        units.append(((nt, 0, t2), v_unit(t2)))
                    return units

                def norm_units(qt, pj, ysbs, coll, rec2):
                    """Normalize head pair pj of query tile qt (divide by the
                    softmax denominators collected in coll)."""
                    q0 = qt * 512
                    units = []

                    def recip():
                        nc.vector.reciprocal_approx_fast(rec2[:], coll[:])
                    units.append((None, recip))

                    def mk(h):
                        def fn():
                            rrow = smallp.tile([1, 512], F32, tag="rrow", bufs=4,
                                               name="rrow")
                            nc.gpsimd.dma_start(rrow[:], rec2[h:h + 1, :])
                            rb = smallp.tile([64, 512], F32, tag="rb", bufs=4,
                                             name="rb")
                            nc.gpsimd.partition_broadcast(rb[:], rrow[:])
                            nc.vector.tensor_mul(
                                yT[pj][64 * h:64 * h + 64, q0:q0 + 512],
                                ysbs[2 * pj + h][0:64, :], rb[:])
                        return fn
                    for h in range(2):
                        units.append((None, mk(h)))
                    return units

                def p3_units(qt):
                    """Projection for token blocks of query tile qt (needs yT
                    cols [512qt, 512qt+512) normalized)."""
                    units = []

                    def mm_unit(tt, of):
                        def fn():
                            ps = auxp.tile([128, 512], F32, tag="aux", name="auxps")
                            for cy in range(4):
                                nc.tensor.matmul(
                                    ps[:], yT[cy][:, tt * 128:(tt + 1) * 128],
                                    wp_t[:, cy * 1024 + of * 512:cy * 1024 + of * 512 + 512],
                                    start=(cy == 0), stop=(cy == 3))
                            o_t = smallp.tile([128, 512], BF16, tag="osb", bufs=3,
                                              name="osb")
                            nc.vector.tensor_copy(o_t[:], ps[:])
                            nc.sync.dma_start(
                                out_d[tt * 128:(tt + 1) * 128,
                                      of * 512:(of + 1) * 512], o_t[:])
                        return fn

                    for tt in range(4 * qt, 4 * qt + 4):
                        for of in range(2):
                            units.append((None, mm_unit(tt, of)))
                    return units

                # ---------- emission ----------
                pending = list(p1_chunk_units(0))

                def emit_due(cur):
                    i = 0
                    while i < len(pending):
                        dl, fn = pending[i]
                        if dl is not None and dl <= cur:
                            fn()
                            pending.pop(i)
                        else:
                            i += 1

                def emit_paced(n):
                    for _ in range(min(n, len(pending))):
                        pending.pop(0)[1]()

                for qt in range(4):
                    q0 = qt * 512
                    ext = 4 * (qt + 1)
                    if qt < 3:
                        pending.extend(p1_chunk_units(qt + 1))

                    ysbs = {}
                    for pj in range(4):
                        coll = smallp.tile([2, 512], F32, tag=f"coll{pj}{qt % 2}",
                                           bufs=1, name="coll")
                        rec2 = smallp.tile([2, 512], F32, tag=f"rec2{pj}{qt % 2}",
                                           bufs=1, name="rec2")
                        y_ps = [ypsp.tile([65, 512], F32, tag=f"yps{h}", bufs=1,
                                          name=f"yps{h}")
                                for h in range(2)]
                        for sc in range(ext):
                            emit_due((qt, pj, sc))
                            r = sc - (ext - 4)
                            off = 0 if r < 0 else 128 * r
                            s_ps = spsp.tile([128, 1024], F32, tag="sps", bufs=2,
                                             name="sps")
                            nc.tensor.matmul(s_ps[:, off:512],
                                             kT[pj][0:64, sc * 128:(sc + 1) * 128],
                                             qT[pj][0:64, q0 + off:q0 + 512],
                                             start=True, stop=True, tile_position=(0, 0))
                            nc.tensor.matmul(s_ps[:, 512 + off:1024],
                                             kT[pj][64:128, sc * 128:(sc + 1) * 128],
                                             qT[pj][64:128, q0 + off:q0 + 512],
                                             start=True, stop=True, tile_position=(64, 0))
                            e_t = ep.tile([128, 1024], BF16, tag="e", bufs=4, name="e")
                            e3 = e_t[:].rearrange("p (h q) -> p h q", h=2)
                            s3 = s_ps[:].rearrange("p (h q) -> p h q", h=2)
                            nc.scalar.activation(e3[:, :, off:512], s3[:, :, off:512],
                                                 AFT.Exp, scale=0.125)
                            if r >= 0:
                                tr3 = tri2_t[:].rearrange("p (h q) -> p h q", h=2)
                                nc.vector.tensor_mul(e3[:, :, off:off + 128],
                                                     e3[:, :, off:off + 128], tr3[:, :, :])
                            for h in range(2):
                                hc = 130 * pj + 65 * h
                                nc.tensor.matmul(y_ps[h][:, off:512],
                                                 vS[sc][:, hc:hc + 65],
                                                 e_t[:, 512 * h + off:512 * h + 512],
                                                 start=(sc == 0), stop=(sc == ext - 1))
                            emit_paced(1)
                        # drain y to SBUF, stash denominators
                        for h in range(2):
                            i = 2 * pj + h
                            y_sb = smallp.tile([65, 512], F32, tag=f"ysb{i}", bufs=1,
                                               name=f"ysb{i}")
                            nc.vector.tensor_copy(y_sb[:], y_ps[h][:])
                            nc.sync.dma_start(coll[h:h + 1, :], y_sb[64:65, :])
                            ysbs[i] = y_sb
                        pending.extend(norm_units(qt, pj, ysbs, coll, rec2))
                    # projection of this qt interleaves into qt+1
                    pending.extend(p3_units(qt))
                # tail
                while pending:
                    pending.pop(0)[1]()

    if not nc.is_finalized():
        nc.finalize()
    return nc


def _get_nc():
    if "nc" not in _CACHE:
        _CACHE["nc"] = build()
    return _CACHE["nc"]


def kernel(x, w_attn, b_attn, w_proj, b_proj, _trace=False, _trace_kwargs=None):
    x = np.asarray(x, dtype=np.float32)
    w_attn = np.asarray(w_attn, dtype=np.float32)
    b_attn = np.asarray(b_attn, dtype=np.float32)
    w_proj = np.asarray(w_proj, dtype=np.float32)
    b_proj = np.asarray(b_proj, dtype=np.float32)

    bf = ml_dtypes.bfloat16

    def pack_w(w):  # [1024, 512] -> [128, 4096] (c-block major columns)
        return np.ascontiguousarray(
            w.reshape(8, 128, 512).transpose(1, 0, 2).reshape(128, 4096)).astype(bf)

    tri = (np.arange(128)[:, None] <= np.arange(128)[None, :]).astype(np.float32)
    tri2 = np.concatenate([tri, tri], axis=1).astype(bf)
    in_maps = []
    for core in range(8):
        b, hg = core // 2, core % 2
        cs = hg * 512
        bq = b_attn[cs:cs + 512]
        bk = b_attn[C + cs:C + cs + 512]
        bqk = np.concatenate([bq.reshape(4, 128).T, bk.reshape(4, 128).T],
                             axis=1).astype(np.float32)
        # xt: [4 chunks, 128, 8*512]: chunk nt, partition p=c_lo, col c*512+t
        xt4 = np.ascontiguousarray(
            x[b].T.reshape(8, 128, 4, 512).transpose(2, 1, 0, 3).reshape(4, 128, 4096)
        ).astype(bf)
        # wp: [512, 1024] -> [128, 4096] (cy-block major)
        wp4 = np.ascontiguousarray(
            w_proj[cs:cs + 512, :].reshape(4, 128, 1024).transpose(1, 0, 2)
            .reshape(128, 4096)).astype(bf)
        in_maps.append({
            "xt": xt4,
            "wq": pack_w(w_attn[:, cs:cs + 512]),
            "wk": pack_w(w_attn[:, C + cs:C + cs + 512]),
            "wv": pack_w(w_attn[:, 2 * C + cs:2 * C + cs + 512]),
            "bqk": bqk,
            "tri2": tri2,
            "wp": wp4,
        })

    kw = {}
    if _trace:
        kw["trace"] = True
        if _trace_kwargs:
            kw.update(_trace_kwargs)
    res = run_bass_kernel_spmd(_get_nc(), in_maps, list(range(8)), **kw)
    _CACHE["last_results"] = res
    outs = [res.results[c]["out"].astype(np.float32) for c in range(8)]
    y = np.stack([outs[2 * b] + outs[2 * b + 1] for b in range(B)])
    beff = (b_proj.astype(np.float64)
            + b_attn[2 * C:].astype(np.float64) @ w_proj.astype(np.float64))
    return (y + beff.reshape(1, 1, C).astype(np.float32)).astype(np.float32)


# revision 8
# speedup vs baseline: 1.2077x; 1.0265x over previous
"""Multi-head causal attention (B=4, T=2048, C=1024, H=16) on 8 trn2 cores.

Sharding: core = (batch b, head-half hg): each core computes QKV for batch b
and its 8 heads, causal attention (scores kept transposed [key, query] so
softmax denominators come from an appended ones-column in V), and a partial
output projection over its 512 y-features. Host sums the two partial
projections per batch and adds b_proj.

Schedule: the scalar engine (exp, ~158us) and tensor engine (~225us) are the
binding resources. P1 (QKV), P3 (proj) and the softmax-denominator normalize
chain are interleaved into P2's per-score-block loop as filler units with
deadlines so the tensor engine runs continuously and never blocks the scalar
exp stream. Diagonal score blocks are trimmed to the causal triangle.
Weights/x are loaded as single consolidated DMAs to minimize trigger cost.
"""

import numpy as np
import ml_dtypes
import concourse.bass as bass
import concourse.mybir as mybir
import concourse.tile as tile
from concourse import bacc
from concourse.bass_utils import run_bass_kernel_spmd

B, T, C = 4, 2048, 1024
H, D = 16, 64
F32 = mybir.dt.float32
BF16 = mybir.dt.bfloat16
AFT = mybir.ActivationFunctionType

_CACHE = {}


def build():
    nc = bacc.Bacc(None, target_bir_lowering=False)
    xt_d = nc.dram_tensor("xt", [4, 128, 4096], BF16, kind="ExternalInput")
    wq_d = nc.dram_tensor("wq", [128, 4096], BF16, kind="ExternalInput")
    wk_d = nc.dram_tensor("wk", [128, 4096], BF16, kind="ExternalInput")
    wv_d = nc.dram_tensor("wv", [128, 4096], BF16, kind="ExternalInput")
    bqk_d = nc.dram_tensor("bqk", [128, 8], F32, kind="ExternalInput")
    tri2_d = nc.dram_tensor("tri2", [128, 256], BF16, kind="ExternalInput")
    wp_d = nc.dram_tensor("wp", [128, 4096], BF16, kind="ExternalInput")
    out_d = nc.dram_tensor("out", [T, C], BF16, kind="ExternalOutput")

    with nc.allow_low_precision(reason="bf16 matmul pipeline"):
        with tile.TileContext(nc) as tc:
            with (
                tc.tile_pool(name="const", bufs=1) as constp,
                tc.tile_pool(name="w1", bufs=1) as w1p,
                tc.tile_pool(name="x", bufs=1) as xp,
                tc.tile_pool(name="qk", bufs=1) as qkp,
                tc.tile_pool(name="vpool", bufs=1) as vp,
                tc.tile_pool(name="esb", bufs=1) as ep,
                tc.tile_pool(name="small", bufs=1) as smallp,
                tc.tile_pool(name="sps", bufs=1, space="PSUM") as spsp,
                tc.tile_pool(name="yps", bufs=1, space="PSUM") as ypsp,
                tc.tile_pool(name="aux", bufs=2, space="PSUM") as auxp,
            ):
                # ---- weights: one consolidated DMA each, spread over queues ----
                wq_t = w1p.tile([128, 4096], BF16, tag="wq")
                nc.sync.dma_start(wq_t[:], wq_d[:])
                wk_t = w1p.tile([128, 4096], BF16, tag="wk")
                nc.gpsimd.dma_start(wk_t[:], wk_d[:])
                wv_t = w1p.tile([128, 4096], BF16, tag="wv")
                nc.gpsimd.dma_start(wv_t[:], wv_d[:])

                bqk_t = constp.tile([128, 8], F32, tag="bqk")
                nc.sync.dma_start(bqk_t[:], bqk_d[:])
                tri2_t = constp.tile([128, 256], BF16, tag="tri2")
                nc.sync.dma_start(tri2_t[:], tri2_d[:])

                wp_t = w1p.tile([128, 4096], BF16, tag="wp")
                nc.gpsimd.dma_start(wp_t[:], wp_d[:])

                # persistent tensors
                qT = [qkp.tile([128, T], BF16, tag=f"qT{j}", name=f"qT{j}") for j in range(4)]
                kT = [qkp.tile([128, T], BF16, tag=f"kT{j}", name=f"kT{j}") for j in range(4)]
                yT = [qkp.tile([128, T], BF16, tag=f"yT{j}", name=f"yT{j}") for j in range(4)]
                vS = [vp.tile([128, 520], BF16, tag=f"v{t}", name=f"v{t}") for t in range(16)]
                # ones column of V (written once; P1 writes only cols 0:64 per head)
                for t in range(16):
                    vv = vS[t][:].rearrange("p (h c) -> p h c", c=65)
                    nc.gpsimd.memset(vv[:, :, 64:65], 1.0)

                # ---------- filler-unit machinery ----------
                # each unit is atomic (opens and closes its own psum group)
                def p1_chunk_units(nt):
                    """P1 for tokens [512*nt, 512*nt+512). Returns list of
                    (deadline, fn); deadline=(qt,pj,sc) = P2 iteration before
                    which the unit must be emitted; None = paced only."""
                    units = []
                    ts0 = nt * 512
                    xt_t = []

                    def dma_x():
                        xx = xp.tile([128, 4096], BF16, tag="xt", bufs=2, name="xx")
                        eng = nc.sync if nt == 0 else nc.gpsimd
                        eng.dma_start(xx[:], xt_d[nt])
                        xt_t.append(xx)
                    units.append(((nt, 0, 0, 0), dma_x))

                    def qk_unit(which, ft):
                        wt = wq_t if which == 0 else wk_t
                        dst = qT[ft] if which == 0 else kT[ft]
                        bcol = ft if which == 0 else 4 + ft

                        def fn():
                            xx = xt_t[0]
                            ps = auxp.tile([128, 512], F32, tag="aux", name="auxps")
                            for c in range(8):
                                nc.tensor.matmul(
                                    ps[:],
                                    wt[:, c * 512 + ft * 128:c * 512 + ft * 128 + 128],
                                    xx[:, c * 512:(c + 1) * 512],
                                    start=(c == 0), stop=(c == 7))
                            nc.vector.tensor_scalar_add(
                                dst[:, ts0:ts0 + 512], ps[:], bqk_t[:, bcol:bcol + 1])
                        return fn

                    def v_unit(t2):
                        def fn():
                            xx = xt_t[0]
                            ps = auxp.tile([128, 512], F32, tag="aux", name="auxps")
                            for c in range(8):
                                nc.tensor.matmul(
                                    ps[:],
                                    xx[:, c * 512 + t2 * 128:c * 512 + t2 * 128 + 128],
                                    wv_t[:, c * 512:(c + 1) * 512],
                                    start=(c == 0), stop=(c == 7))
                            vv = vS[nt * 4 + t2][:].rearrange("p (h c) -> p h c", c=65)
                            nc.vector.tensor_copy(
                                vv[:, :, 0:64],
                                ps[:].rearrange("p (h c) -> p h c", c=64))
                        return fn

                    for ft in range(4):
                        units.append(((nt, ft, 0, 0), qk_unit(0, ft)))
                        units.append(((nt, ft, 0, 0), qk_unit(1, ft)))
                    for t2 in range(4):
                        units.append(((nt, 0, t2, 1), v_unit(t2)))
                    return units

                def norm_units(qt, pj, ysbs, coll, rec2):
                    """Normalize head pair pj of query tile qt (divide by the
                    softmax denominators collected in coll)."""
                    q0 = qt * 512
                    units = []

                    def recip():
                        nc.vector.reciprocal_approx_fast(rec2[:], coll[:])
                    units.append((None, recip))

                    def mk(h):
                        def fn():
                            if h == 0:
                                srow = rec2[0:1, :]
                            else:
                                srow = smallp.tile([1, 512], F32, tag="rrow",
                                                   bufs=4, name="rrow")
                                nc.sync.dma_start(srow[:], rec2[h:h + 1, :])
                                srow = srow[:]
                            rb = smallp.tile([64, 512], F32, tag="rb", bufs=4,
                                             name="rb")
                            nc.gpsimd.partition_broadcast(rb[:], srow)
                            nc.vector.tensor_mul(
                                yT[pj][64 * h:64 * h + 64, q0:q0 + 512],
                                ysbs[2 * pj + h][0:64, :], rb[:])
                        return fn
                    for h in range(2):
                        units.append((None, mk(h)))
                    return units

                def p3_units(qt):
                    """Projection for token blocks of query tile qt (needs yT
                    cols [512qt, 512qt+512) normalized)."""
                    units = []

                    def mm_unit(tt, of):
                        def fn():
                            ps = auxp.tile([128, 512], F32, tag="aux", name="auxps")
                            for cy in range(4):
                                nc.tensor.matmul(
                                    ps[:], yT[cy][:, tt * 128:(tt + 1) * 128],
                                    wp_t[:, cy * 1024 + of * 512:cy * 1024 + of * 512 + 512],
                                    start=(cy == 0), stop=(cy == 3))
                            o_t = smallp.tile([128, 512], BF16, tag="osb", bufs=3,
                                              name="osb")
                            nc.vector.tensor_copy(o_t[:], ps[:])
                            nc.sync.dma_start(
                                out_d[tt * 128:(tt + 1) * 128,
                                      of * 512:(of + 1) * 512], o_t[:])
                        return fn

                    for tt in range(4 * qt, 4 * qt + 4):
                        for of in range(2):
                            units.append((None, mm_unit(tt, of)))
                    return units

                def p3a_units(qt, parts):
                    """First 3/4 of the qt projection (heads of pj 0-2), kept
                    in SBUF so only the pj3 contribution remains at the tail."""
                    units = []

                    def mm_unit(tt, of):
                        def fn():
                            ps = auxp.tile([128, 512], F32, tag="aux", name="auxps")
                            for cy in range(3):
                                nc.tensor.matmul(
                                    ps[:], yT[cy][:, tt * 128:(tt + 1) * 128],
                                    wp_t[:, cy * 1024 + of * 512:cy * 1024 + of * 512 + 512],
                                    start=(cy == 0), stop=(cy == 2))
                            o_p = smallp.tile([128, 512], F32, tag=f"opart{(tt % 4) * 2 + of}",
                                              bufs=1, name="opart")
                            nc.vector.tensor_copy(o_p[:], ps[:])
                            parts[(tt, of)] = o_p
                        return fn

                    for tt in range(4 * qt, 4 * qt + 4):
                        for of in range(2):
                            units.append((None, mm_unit(tt, of)))
                    return units

                def p3b_units(qt, parts):
                    units = []

                    def mm_unit(tt, of):
                        def fn():
                            ps = auxp.tile([128, 512], F32, tag="aux", name="auxps")
                            nc.tensor.matmul(
                                ps[:], yT[3][:, tt * 128:(tt + 1) * 128],
                                wp_t[:, 3 * 1024 + of * 512:3 * 1024 + of * 512 + 512],
                                start=True, stop=True)
                            o_t = smallp.tile([128, 512], BF16, tag="osb", bufs=3,
                                              name="osb")
                            nc.vector.tensor_add(o_t[:], ps[:], parts[(tt, of)][:])
                            nc.sync.dma_start(
                                out_d[tt * 128:(tt + 1) * 128,
                                      of * 512:(of + 1) * 512], o_t[:])
                        return fn

                    for tt in range(4 * qt, 4 * qt + 4):
                        for of in range(2):
                            units.append((None, mm_unit(tt, of)))
                    return units

                # ---------- emission ----------
                pending = list(p1_chunk_units(0))

                def emit_due(cur):
                    i = 0
                    while i < len(pending):
                        dl, fn = pending[i]
                        if dl is not None and dl <= cur:
                            fn()
                            pending.pop(i)
                        else:
                            i += 1

                def emit_paced(n):
                    for _ in range(min(n, len(pending))):
                        pending.pop(0)[1]()

                p3parts = {}
                for qt in range(4):
                    q0 = qt * 512
                    ext = 4 * (qt + 1)
                    if qt < 3:
                        pending.extend(p1_chunk_units(qt + 1))

                    ysbs = {}
                    for pj in range(4):
                        coll = smallp.tile([2, 512], F32, tag=f"coll{pj}{qt % 2}",
                                           bufs=1, name="coll")
                        rec2 = smallp.tile([2, 512], F32, tag=f"rec2{pj}{qt % 2}",
                                           bufs=1, name="rec2")
                        y_ps = [ypsp.tile([65, 512], F32, tag=f"yps{h}", bufs=1,
                                          name=f"yps{h}")
                                for h in range(2)]
                        for sc in range(ext):
                            emit_due((qt, pj, sc, 0))
                            r = sc - (ext - 4)
                            off = 0 if r < 0 else 128 * r
                            s_ps = spsp.tile([128, 1024], F32, tag="sps", bufs=2,
                                             name="sps")
                            nc.tensor.matmul(s_ps[:, off:512],
                                             kT[pj][0:64, sc * 128:(sc + 1) * 128],
                                             qT[pj][0:64, q0 + off:q0 + 512],
                                             start=True, stop=True, tile_position=(0, 0))
                            nc.tensor.matmul(s_ps[:, 512 + off:1024],
                                             kT[pj][64:128, sc * 128:(sc + 1) * 128],
                                             qT[pj][64:128, q0 + off:q0 + 512],
                                             start=True, stop=True, tile_position=(64, 0))
                            e_t = ep.tile([128, 1024], BF16, tag="e", bufs=4, name="e")
                            e3 = e_t[:].rearrange("p (h q) -> p h q", h=2)
                            s3 = s_ps[:].rearrange("p (h q) -> p h q", h=2)
                            nc.scalar.activation(e3[:, :, off:512], s3[:, :, off:512],
                                                 AFT.Exp, scale=0.125)
                            emit_due((qt, pj, sc, 1))
                            if r >= 0:
                                tr3 = tri2_t[:].rearrange("p (h q) -> p h q", h=2)
                                nc.vector.tensor_mul(e3[:, :, off:off + 128],
                                                     e3[:, :, off:off + 128], tr3[:, :, :])
                            for h in range(2):
                                hc = 130 * pj + 65 * h
                                nc.tensor.matmul(y_ps[h][:, off:512],
                                                 vS[sc][:, hc:hc + 65],
                                                 e_t[:, 512 * h + off:512 * h + 512],
                                                 start=(sc == 0), stop=(sc == ext - 1))
                            emit_paced(1)
                        # drain y to SBUF, stash denominators
                        for h in range(2):
                            i = 2 * pj + h
                            y_sb = smallp.tile([65, 512], F32, tag=f"ysb{i}", bufs=1,
                                               name=f"ysb{i}")
                            nc.vector.tensor_copy(y_sb[:], y_ps[h][:])
                            nc.sync.dma_start(coll[h:h + 1, :], y_sb[64:65, :])
                            ysbs[i] = y_sb
                        pending.extend(norm_units(qt, pj, ysbs, coll, rec2))
                        if qt == 3 and pj == 2:
                            pending.extend(p3a_units(3, p3parts))
                    # projection of this qt interleaves into qt+1
                    if qt < 3:
                        pending.extend(p3_units(qt))
                    else:
                        pending.extend(p3b_units(3, p3parts))
                # tail
                while pending:
                    pending.pop(0)[1]()

    if not nc.is_finalized():
        nc.finalize()
    return nc


def _get_nc():
    if "nc" not in _CACHE:
        _CACHE["nc"] = build()
    return _CACHE["nc"]


def kernel(x, w_attn, b_attn, w_proj, b_proj, _trace=False, _trace_kwargs=None):
    x = np.asarray(x, dtype=np.float32)
    w_attn = np.asarray(w_attn, dtype=np.float32)
    b_attn = np.asarray(b_attn, dtype=np.float32)
    w_proj = np.asarray(w_proj, dtype=np.float32)
    b_proj = np.asarray(b_proj, dtype=np.float32)

    bf = ml_dtypes.bfloat16

    def pack_w(w):  # [1024, 512] -> [128, 4096] (c-block major columns)
        return np.ascontiguousarray(
            w.reshape(8, 128, 512).transpose(1, 0, 2).reshape(128, 4096)).astype(bf)

    tri = (np.arange(128)[:, None] <= np.arange(128)[None, :]).astype(np.float32)
    tri2 = np.concatenate([tri, tri], axis=1).astype(bf)
    in_maps = []
    for core in range(8):
        b, hg = core // 2, core % 2
        cs = hg * 512
        bq = b_attn[cs:cs + 512]
        bk = b_attn[C + cs:C + cs + 512]
        bqk = np.concatenate([bq.reshape(4, 128).T, bk.reshape(4, 128).T],
                             axis=1).astype(np.float32)
        # xt: [4 chunks, 128, 8*512]: chunk nt, partition p=c_lo, col c*512+t
        xt4 = np.ascontiguousarray(
            x[b].T.reshape(8, 128, 4, 512).transpose(2, 1, 0, 3).reshape(4, 128, 4096)
        ).astype(bf)
        # wp: [512, 1024] -> [128, 4096] (cy-block major)
        wp4 = np.ascontiguousarray(
            w_proj[cs:cs + 512, :].reshape(4, 128, 1024).transpose(1, 0, 2)
            .reshape(128, 4096)).astype(bf)
        in_maps.append({
            "xt": xt4,
            "wq": pack_w(w_attn[:, cs:cs + 512]),
            "wk": pack_w(w_attn[:, C + cs:C + cs + 512]),
            "wv": pack_w(w_attn[:, 2 * C + cs:2 * C + cs + 512]),
            "bqk": bqk,
            "tri2": tri2,
            "wp": wp4,
        })

    kw = {}
    if _trace:
        kw["trace"] = True
        if _trace_kwargs:
            kw.update(_trace_kwargs)
    res = run_bass_kernel_spmd(_get_nc(), in_maps, list(range(8)), **kw)
    _CACHE["last_results"] = res
    outs = [res.results[c]["out"].astype(np.float32) for c in range(8)]
    y = np.stack([outs[2 * b] + outs[2 * b + 1] for b in range(B)])
    beff = (b_proj.astype(np.float64)
            + b_attn[2 * C:].astype(np.float64) @ w_proj.astype(np.float64))
    return (y + beff.reshape(1, 1, C).astype(np.float32)).astype(np.float32)


# revision 10
# speedup vs baseline: 1.2174x; 1.0080x over previous
"""Multi-head causal attention (B=4, T=2048, C=1024, H=16) on 8 trn2 cores.

Sharding: core = (batch b, head-half hg): each core computes QKV for batch b
and its 8 heads, causal attention (scores kept transposed [key, query] so
softmax denominators come from an appended ones-column in V), and a partial
output projection over its 512 y-features. Host sums the two partial
projections per batch and adds b_proj.

Schedule: the scalar engine (exp, ~158us) and tensor engine (~225us) are the
binding resources. P1 (QKV), P3 (proj) and the softmax-denominator normalize
chain are interleaved into P2's per-score-block loop as filler units with
deadlines so the tensor engine runs continuously and never blocks the scalar
exp stream. Diagonal score blocks are trimmed to the causal triangle.
Weights/x are loaded as single consolidated DMAs to minimize trigger cost.
"""

import numpy as np
import ml_dtypes
import concourse.bass as bass
import concourse.mybir as mybir
import concourse.tile as tile
from concourse import bacc
from concourse.bass_utils import run_bass_kernel_spmd

B, T, C = 4, 2048, 1024
H, D = 16, 64
F32 = mybir.dt.float32
BF16 = mybir.dt.bfloat16
AFT = mybir.ActivationFunctionType

_CACHE = {}


def build():
    nc = bacc.Bacc(None, target_bir_lowering=False)
    xt_d = nc.dram_tensor("xt", [4, 128, 4096], BF16, kind="ExternalInput")
    wq_d = nc.dram_tensor("wq", [128, 4096], BF16, kind="ExternalInput")
    wk_d = nc.dram_tensor("wk", [128, 4096], BF16, kind="ExternalInput")
    wv_d = nc.dram_tensor("wv", [128, 4096], BF16, kind="ExternalInput")
    bqk_d = nc.dram_tensor("bqk", [128, 8], F32, kind="ExternalInput")
    tri2_d = nc.dram_tensor("tri2", [128, 256], BF16, kind="ExternalInput")
    wp_d = nc.dram_tensor("wp", [128, 4096], BF16, kind="ExternalInput")
    out_d = nc.dram_tensor("out", [T, C], BF16, kind="ExternalOutput")

    with nc.allow_low_precision(reason="bf16 matmul pipeline"):
        with tile.TileContext(nc) as tc:
            with (
                tc.tile_pool(name="const", bufs=1) as constp,
                tc.tile_pool(name="w1", bufs=1) as w1p,
                tc.tile_pool(name="x", bufs=1) as xp,
                tc.tile_pool(name="qk", bufs=1) as qkp,
                tc.tile_pool(name="vpool", bufs=1) as vp,
                tc.tile_pool(name="esb", bufs=1) as ep,
                tc.tile_pool(name="small", bufs=1) as smallp,
                tc.tile_pool(name="sps", bufs=1, space="PSUM") as spsp,
                tc.tile_pool(name="yps", bufs=1, space="PSUM") as ypsp,
                tc.tile_pool(name="aux", bufs=2, space="PSUM") as auxp,
            ):
                # ---- startup loads: first-needed pieces split over all 3
                # DMA-capable queues (wq/wk are packed ft-major by the host) ----
                bqk_t = constp.tile([128, 8], F32, tag="bqk")
                nc.sync.dma_start(bqk_t[:], bqk_d[:])
                tri2_t = constp.tile([128, 256], BF16, tag="tri2")
                nc.sync.dma_start(tri2_t[:], tri2_d[:])

                wq_t = w1p.tile([128, 4096], BF16, tag="wq")
                wk_t = w1p.tile([128, 4096], BF16, tag="wk")
                wv_t = w1p.tile([128, 4096], BF16, tag="wv")
                wp_t = w1p.tile([128, 4096], BF16, tag="wp")
                x0 = xp.tile([128, 4096], BF16, tag="xt", bufs=2, name="x0")
                nc.sync.dma_start(x0[:, 0:1536], xt_d[0, :, 0:1536])
                nc.scalar.dma_start(x0[:, 1536:3072], xt_d[0, :, 1536:3072])
                nc.gpsimd.dma_start(x0[:, 3072:4096], xt_d[0, :, 3072:4096])
                for ft in range(4):
                    nc.sync.dma_start(wq_t[:, ft * 1024:(ft + 1) * 1024],
                                      wq_d[:, ft * 1024:(ft + 1) * 1024])
                    nc.scalar.dma_start(wk_t[:, ft * 1024:(ft + 1) * 1024],
                                        wk_d[:, ft * 1024:(ft + 1) * 1024])
                nc.gpsimd.dma_start(wv_t[:, 0:2048], wv_d[:, 0:2048])
                nc.scalar.dma_start(wv_t[:, 2048:4096], wv_d[:, 2048:4096])
                nc.gpsimd.dma_start(wp_t[:], wp_d[:])

                # persistent tensors
                qT = [qkp.tile([128, T], BF16, tag=f"qT{j}", name=f"qT{j}") for j in range(4)]
                kT = [qkp.tile([128, T], BF16, tag=f"kT{j}", name=f"kT{j}") for j in range(4)]
                yT = [qkp.tile([128, T], BF16, tag=f"yT{j}", name=f"yT{j}") for j in range(4)]
                vS = [vp.tile([128, 520], BF16, tag=f"v{t}", name=f"v{t}") for t in range(16)]
                # ones column of V (written once; P1 writes only cols 0:64 per head)
                for t in range(16):
                    vv = vS[t][:].rearrange("p (h c) -> p h c", c=65)
                    nc.gpsimd.memset(vv[:, :, 64:65], 1.0)

                # ---------- filler-unit machinery ----------
                # each unit is atomic (opens and closes its own psum group)
                def p1_chunk_units(nt):
                    """P1 for tokens [512*nt, 512*nt+512). Returns list of
                    (deadline, fn); deadline=(qt,pj,sc) = P2 iteration before
                    which the unit must be emitted; None = paced only."""
                    units = []
                    ts0 = nt * 512
                    xt_t = []

                    def dma_x():
                        if nt == 0:
                            xt_t.append(x0)
                            return
                        xx = xp.tile([128, 4096], BF16, tag="xt", bufs=2, name="xx")
                        nc.gpsimd.dma_start(xx[:], xt_d[nt])
                        xt_t.append(xx)
                    units.append(((nt, 0, 0, 0), dma_x))

                    def qk_unit(which, ft):
                        wt = wq_t if which == 0 else wk_t
                        dst = qT[ft] if which == 0 else kT[ft]
                        bcol = ft if which == 0 else 4 + ft

                        def fn():
                            xx = xt_t[0]
                            ps = auxp.tile([128, 512], F32, tag="aux", name="auxps")
                            for c in range(8):
                                nc.tensor.matmul(
                                    ps[:],
                                    wt[:, ft * 1024 + c * 128:ft * 1024 + c * 128 + 128],
                                    xx[:, c * 512:(c + 1) * 512],
                                    start=(c == 0), stop=(c == 7))
                            nc.vector.tensor_scalar_add(
                                dst[:, ts0:ts0 + 512], ps[:], bqk_t[:, bcol:bcol + 1])
                        return fn

                    def v_unit(t2):
                        def fn():
                            xx = xt_t[0]
                            ps = auxp.tile([128, 512], F32, tag="aux", name="auxps")
                            for c in range(8):
                                nc.tensor.matmul(
                                    ps[:],
                                    xx[:, c * 512 + t2 * 128:c * 512 + t2 * 128 + 128],
                                    wv_t[:, c * 512:(c + 1) * 512],
                                    start=(c == 0), stop=(c == 7))
                            vv = vS[nt * 4 + t2][:].rearrange("p (h c) -> p h c", c=65)
                            nc.vector.tensor_copy(
                                vv[:, :, 0:64],
                                ps[:].rearrange("p (h c) -> p h c", c=64))
                        return fn

                    for ft in range(4):
                        units.append(((nt, ft, 0, 0), qk_unit(0, ft)))
                        units.append(((nt, ft, 0, 0), qk_unit(1, ft)))
                    for t2 in range(4):
                        units.append(((nt, 0, t2, 1), v_unit(t2)))
                    return units

                def norm_units(qt, pj, ysbs, coll, rec2):
                    """Normalize head pair pj of query tile qt (divide by the
                    softmax denominators collected in coll)."""
                    q0 = qt * 512
                    units = []

                    def recip():
                        nc.vector.reciprocal_approx_fast(rec2[:], coll[:])
                    units.append((None, recip))

                    def mk(h):
                        def fn():
                            if h == 0:
                                srow = rec2[0:1, :]
                            else:
                                srow = smallp.tile([1, 512], F32, tag="rrow",
                                                   bufs=4, name="rrow")
                                nc.sync.dma_start(srow[:], rec2[h:h + 1, :])
                                srow = srow[:]
                            rb = smallp.tile([64, 512], F32, tag="rb", bufs=4,
                                             name="rb")
                            nc.gpsimd.partition_broadcast(rb[:], srow)
                            nc.vector.tensor_mul(
                                yT[pj][64 * h:64 * h + 64, q0:q0 + 512],
                                ysbs[2 * pj + h][0:64, :], rb[:])
                        return fn
                    for h in range(2):
                        units.append((None, mk(h)))
                    return units

                def p3_units(qt):
                    """Projection for token blocks of query tile qt (needs yT
                    cols [512qt, 512qt+512) normalized)."""
                    units = []

                    def mm_unit(tt, of):
                        def fn():
                            ps = auxp.tile([128, 512], F32, tag="aux", name="auxps")
                            for cy in range(4):
                                nc.tensor.matmul(
                                    ps[:], yT[cy][:, tt * 128:(tt + 1) * 128],
                                    wp_t[:, cy * 1024 + of * 512:cy * 1024 + of * 512 + 512],
                                    start=(cy == 0), stop=(cy == 3))
                            o_t = smallp.tile([128, 512], BF16, tag="osb", bufs=3,
                                              name="osb")
                            nc.vector.tensor_copy(o_t[:], ps[:])
                            nc.sync.dma_start(
                                out_d[tt * 128:(tt + 1) * 128,
                                      of * 512:(of + 1) * 512], o_t[:])
                        return fn

                    for tt in range(4 * qt, 4 * qt + 4):
                        for of in range(2):
                            units.append((None, mm_unit(tt, of)))
                    return units

                def p3a_units(qt, parts):
                    """First 3/4 of the qt projection (heads of pj 0-2), kept
                    in SBUF so only the pj3 contribution remains at the tail."""
                    units = []

                    def mm_unit(tt, of):
                        def fn():
                            ps = auxp.tile([128, 512], F32, tag="aux", name="auxps")
                            for cy in range(3):
                                nc.tensor.matmul(
                                    ps[:], yT[cy][:, tt * 128:(tt + 1) * 128],
                                    wp_t[:, cy * 1024 + of * 512:cy * 1024 + of * 512 + 512],
                                    start=(cy == 0), stop=(cy == 2))
                            o_p = smallp.tile([128, 512], F32, tag=f"opart{(tt % 4) * 2 + of}",
                                              bufs=1, name="opart")
                            nc.vector.tensor_copy(o_p[:], ps[:])
                            parts[(tt, of)] = o_p
                        return fn

                    for tt in range(4 * qt, 4 * qt + 4):
                        for of in range(2):
                            units.append((None, mm_unit(tt, of)))
                    return units

                def p3b_units(qt, parts):
                    units = []

                    def mm_unit(tt, of):
                        def fn():
                            ps = auxp.tile([128, 512], F32, tag="aux", name="auxps")
                            nc.tensor.matmul(
                                ps[:], yT[3][:, tt * 128:(tt + 1) * 128],
                                wp_t[:, 3 * 1024 + of * 512:3 * 1024 + of * 512 + 512],
                                start=True, stop=True)
                            o_t = smallp.tile([128, 512], BF16, tag="osb", bufs=3,
                                              name="osb")
                            nc.vector.tensor_add(o_t[:], ps[:], parts[(tt, of)][:])
                            nc.sync.dma_start(
                                out_d[tt * 128:(tt + 1) * 128,
                                      of * 512:(of + 1) * 512], o_t[:])
                        return fn

                    for tt in range(4 * qt, 4 * qt + 4):
                        for of in range(2):
                            units.append((None, mm_unit(tt, of)))
                    return units

                # ---------- emission ----------
                pending = list(p1_chunk_units(0))

                def emit_due(cur):
                    i = 0
                    while i < len(pending):
                        dl, fn = pending[i]
                        if dl is not None and dl <= cur:
                            fn()
                            pending.pop(i)
                        else:
                            i += 1

                def emit_paced(n):
                    for _ in range(min(n, len(pending))):
                        pending.pop(0)[1]()

                p3parts = {}
                for qt in range(4):
                    q0 = qt * 512
                    ext = 4 * (qt + 1)
                    if qt < 3:
                        pending.extend(p1_chunk_units(qt + 1))

                    ysbs = {}
                    for pj in range(4):
                        coll = smallp.tile([2, 512], F32, tag=f"coll{pj}{qt % 2}",
                                           bufs=1, name="coll")
                        rec2 = smallp.tile([2, 512], F32, tag=f"rec2{pj}{qt % 2}",
                                           bufs=1, name="rec2")
                        y_ps = [ypsp.tile([65, 512], F32, tag=f"yps{h}", bufs=1,
                                          name=f"yps{h}")
                                for h in range(2)]
                        for sc in range(ext):
                            emit_due((qt, pj, sc, 0))
                            r = sc - (ext - 4)
                            off = 0 if r < 0 else 128 * r
                            s_ps = spsp.tile([128, 1024], F32, tag="sps", bufs=2,
                                             name="sps")
                            nc.tensor.matmul(s_ps[:, off:512],
                                             kT[pj][0:64, sc * 128:(sc + 1) * 128],
                                             qT[pj][0:64, q0 + off:q0 + 512],
                                             start=True, stop=True, tile_position=(0, 0))
                            nc.tensor.matmul(s_ps[:, 512 + off:1024],
                                             kT[pj][64:128, sc * 128:(sc + 1) * 128],
                                             qT[pj][64:128, q0 + off:q0 + 512],
                                             start=True, stop=True, tile_position=(64, 0))
                            e_t = ep.tile([128, 1024], BF16, tag="e", bufs=4, name="e")
                            e3 = e_t[:].rearrange("p (h q) -> p h q", h=2)
                            s3 = s_ps[:].rearrange("p (h q) -> p h q", h=2)
                            nc.scalar.activation(e3[:, :, off:512], s3[:, :, off:512],
                                                 AFT.Exp, scale=0.125)
                            emit_due((qt, pj, sc, 1))
                            if r >= 0:
                                tr3 = tri2_t[:].rearrange("p (h q) -> p h q", h=2)
                                nc.vector.tensor_mul(e3[:, :, off:off + 128],
                                                     e3[:, :, off:off + 128], tr3[:, :, :])
                            for h in range(2):
                                hc = 130 * pj + 65 * h
                                nc.tensor.matmul(y_ps[h][:, off:512],
                                                 vS[sc][:, hc:hc + 65],
                                                 e_t[:, 512 * h + off:512 * h + 512],
                                                 start=(sc == 0), stop=(sc == ext - 1))
                            emit_paced(1)
                        # drain y to SBUF, stash denominators
                        for h in range(2):
                            i = 2 * pj + h
                            y_sb = smallp.tile([65, 512], F32, tag=f"ysb{i}", bufs=1,
                                               name=f"ysb{i}")
                            nc.vector.tensor_copy(y_sb[:], y_ps[h][:])
                            nc.sync.dma_start(coll[h:h + 1, :], y_sb[64:65, :])
                            ysbs[i] = y_sb
                        pending.extend(norm_units(qt, pj, ysbs, coll, rec2))
                        if qt == 3 and pj == 2:
                            pending.extend(p3a_units(3, p3parts))
                    # projection of this qt interleaves into qt+1
                    if qt < 3:
                        pending.extend(p3_units(qt))
                    else:
                        pending.extend(p3b_units(3, p3parts))
                # tail
                while pending:
                    pending.pop(0)[1]()

    if not nc.is_finalized():
        nc.finalize()
    return nc


def _get_nc():
    if "nc" not in _CACHE:
        _CACHE["nc"] = build()
    return _CACHE["nc"]


def kernel(x, w_attn, b_attn, w_proj, b_proj, _trace=False, _trace_kwargs=None):
    x = np.asarray(x, dtype=np.float32)
    w_attn = np.asarray(w_attn, dtype=np.float32)
    b_attn = np.asarray(b_attn, dtype=np.float32)
    w_proj = np.asarray(w_proj, dtype=np.float32)
    b_proj = np.asarray(b_proj, dtype=np.float32)

    bf = ml_dtypes.bfloat16

    def pack_w(w):  # [1024, 512] -> [128, 4096] (c-block major columns)
        return np.ascontiguousarray(
            w.reshape(8, 128, 512).transpose(1, 0, 2).reshape(128, 4096)).astype(bf)

    def pack_w_ft(w):  # [1024, 512] -> [128, 4096] (ft-major: ft*1024 + c*128 + j)
        # w[c*128+p, ft*128+j] -> out[p, ft*1024 + c*128 + j]
        return np.ascontiguousarray(
            w.reshape(8, 128, 4, 128).transpose(1, 2, 0, 3).reshape(128, 4096)
        ).astype(bf)

    tri = (np.arange(128)[:, None] <= np.arange(128)[None, :]).astype(np.float32)
    tri2 = np.concatenate([tri, tri], axis=1).astype(bf)
    in_maps = []
    for core in range(8):
        b, hg = core // 2, core % 2
        cs = hg * 512
        bq = b_attn[cs:cs + 512]
        bk = b_attn[C + cs:C + cs + 512]
        bqk = np.concatenate([bq.reshape(4, 128).T, bk.reshape(4, 128).T],
                             axis=1).astype(np.float32)
        # xt: [4 chunks, 128, 8*512]: chunk nt, partition p=c_lo, col c*512+t
        xt4 = np.ascontiguousarray(
            x[b].T.reshape(8, 128, 4, 512).transpose(2, 1, 0, 3).reshape(4, 128, 4096)
        ).astype(bf)
        # wp: [512, 1024] -> [128, 4096] (cy-block major)
        wp4 = np.ascontiguousarray(
            w_proj[cs:cs + 512, :].reshape(4, 128, 1024).transpose(1, 0, 2)
            .reshape(128, 4096)).astype(bf)
        in_maps.append({
            "xt": xt4,
            "wq": pack_w_ft(w_attn[:, cs:cs + 512]),
            "wk": pack_w_ft(w_attn[:, C + cs:C + cs + 512]),
            "wv": pack_w(w_attn[:, 2 * C + cs:2 * C + cs + 512]),
            "bqk": bqk,
            "tri2": tri2,
            "wp": wp4,
        })

    kw = {}
    if _trace:
        kw["trace"] = True
        if _trace_kwargs:
            kw.update(_trace_kwargs)
    res = run_bass_kernel_spmd(_get_nc(), in_maps, list(range(8)), **kw)
    _CACHE["last_results"] = res
    outs = [res.results[c]["out"].astype(np.float32) for c in range(8)]
    y = np.stack([outs[2 * b] + outs[2 * b + 1] for b in range(B)])
    beff = (b_proj.astype(np.float64)
            + b_attn[2 * C:].astype(np.float64) @ w_proj.astype(np.float64))
    return (y + beff.reshape(1, 1, C).astype(np.float32)).astype(np.float32)
